# revision 2
# baseline (speedup 1.0000x reference)
"""Trainium2 Bass kernel for CompressedLinear (VQ codebook linear layer).

Computes: out = x @ W^T + bias, where
  W = (centroids[indices] @ Pi) * row_norms[:, None]

Upload-minimizing SPMD layout (8 cores):
  - x token-sharded:  core c gets xT[:, c*1024:(c+1)*1024]   (8 MB bf16)
  - Pi column-sharded: core c gets Pi[:, c*512:(c+1)*512]    (4 MB bf16)
  - indices o-sharded: core c gets idxT[:, c*512:(c+1)*512]  (2 MB fp8)
  - rn/bias replicated (32 KB)
Device pipeline per core:
  0. AllGather Pi column shards -> full Pi (32 MB, on-chip links).
  1. DVE gather yts[j,o] = centroids[idxT[j,o]] for the core's o-shard
     (telescoping sum of is_ge masks; fp8 index compare).
  2. PE: W^T[i, o_shard] = sum_j Pi[j,i] * yts[j,o] for all 32 i-blocks.
  3. AllGather W^T o-shards (4 o-quarter chunks) -> full W^T.
  4. PE: outT[o, t_c] = rn[o] * sum_i W^T[i,o] * xT[i,t_c] + bias[o]
     for ALL output features o, local tokens t_c.
Host reassembles the 8 token-shards of outT.
"""

import numpy as np

# Problem geometry (hardcoded per contract)
OUT, IN = 4096, 4096
B, S = 4, 2048
T = B * S          # 8192 tokens
NCORES = 8
P = 128            # partitions
TSH = T // NCORES  # 1024 tokens per core
OSH = OUT // NCORES  # 512 out-features per core (gather/W stage)
NI = IN // P       # 32 i-blocks
NJ = IN // P       # 32 j-blocks
NOB = OUT // P     # 32 o-blocks (stage 4)

_NC_CACHE = {}


def build_nc(cvals):
    """Build the SPMD Bass program. cvals: 16 python floats (codebook)."""
    import concourse.bacc as bacc
    import concourse.mybir as mybir
    from concourse.tile import TileContext

    f32 = mybir.dt.float32
    bf16 = mybir.dt.bfloat16
    fp8 = mybir.dt.float8e4

    rg = [list(range(NCORES))]
    jg = 4             # j-blocks per gather group
    ngr = NJ // jg     # gather groups
    igrp = 4           # i-blocks per stage-2 group
    nig = NI // igrp
    tch = 512          # stage-4 token chunk
    ntc = TSH // tch   # 2

    nc = bacc.Bacc()
    xT_d = nc.dram_tensor("xT", [IN, TSH], bf16, kind="ExternalInput")
    piC_d = nc.dram_tensor("piC", [igrp, IN, P], bf16, kind="ExternalInput")
    idx_d = nc.dram_tensor("idx8", [IN, OSH], fp8, kind="ExternalInput")
    rn_d = nc.dram_tensor("rn", [OUT], f32, kind="ExternalInput")
    bias_d = nc.dram_tensor("bias", [OUT], f32, kind="ExternalInput")
    outT_d = nc.dram_tensor("outT", [OUT, TSH], f32, kind="ExternalOutput")

    piB = nc.dram_tensor("piB", [igrp, IN, P], bf16)
    gathPi = nc.dram_tensor("gathPi", [NI, IN, P], bf16, addr_space="Shared")
    wtbq = [nc.dram_tensor(f"wtb{q}", [NI, P, P], bf16) for q in range(4)]
    gWTq = [
        nc.dram_tensor(f"gWT{q}", [NCORES, NI, P, P], bf16, addr_space="Shared")
        for q in range(4)
    ]

    with TileContext(nc) as tc:
        with (
            tc.tile_pool(name="constp", bufs=1) as constp,
            tc.tile_pool(name="idxp", bufs=2) as idxp,
            tc.tile_pool(name="ytsp", bufs=1) as ytsp,
            tc.tile_pool(name="pip", bufs=igrp + 1) as pip,
            tc.tile_pool(name="wtp", bufs=4) as wtp,
            tc.tile_pool(name="xtp", bufs=1) as xtp,
            tc.tile_pool(name="wt3p", bufs=2) as wt3p,
            tc.tile_pool(name="outp", bufs=4) as outp,
            tc.tile_pool(name="wpsum", bufs=1, space="PSUM") as wpsum,
            tc.tile_pool(name="mpsum", bufs=2, space="PSUM") as mpsum,
        ):
            rn_sb = constp.tile([P, NOB], f32, name="rn_sb")
            nc.sync.dma_start(rn_sb[:], rn_d.rearrange("(b p) -> p b", p=P))
            bias_sb = constp.tile([P, NOB], f32, name="bias_sb")
            nc.sync.dma_start(bias_sb[:], bias_d.rearrange("(b p) -> p b", p=P))

            # ---- Stage 0: AllGather Pi (starts immediately) ---------------
            nc.sync.dma_start(piB[:], piC_d[:])
            nc.gpsimd.collective_compute(
                "AllGather", mybir.AluOpType.bypass, replica_groups=rg,
                ins=[piB[:]], outs=[gathPi[:]],
            )

            # Resident xT for stage 4 (local input, load up front)
            xt = xtp.tile([P, NI, TSH], bf16, name="xt")
            nc.sync.dma_start(
                xt[:], xT_d.rearrange("(a p) t -> p a t", p=P)
            )

            # ---- Stage 1: codebook gather: yts[g][p, b, o] ----------------
            # Telescoping: c[idx] = c0 + sum_{k=1..15} (idx>=k)*(ck-ck-1)
            dk = [float(cvals[k] - cvals[k - 1]) for k in range(1, 16)]
            yts = []
            for g in range(ngr):
                idx_t = idxp.tile([P, jg, OSH], fp8, name="idx_t", tag="idx")
                nc.sync.dma_start(
                    idx_t[:],
                    idx_d[g * jg * P:(g + 1) * jg * P, :].rearrange(
                        "(a p) o -> p a o", p=P
                    ),
                )
                acc = idxp.tile([P, jg, OSH], f32, name="acc", tag="acc")
                nc.vector.tensor_scalar(acc[:], idx_t[:], 1.0, dk[0],
                                        mybir.AluOpType.is_ge,
                                        mybir.AluOpType.mult)
                tmp = idxp.tile([P, jg, OSH], f32, name="tmp", tag="tmp")
                for k in range(2, 16):
                    nc.vector.tensor_scalar(tmp[:], idx_t[:], float(k),
                                            dk[k - 1],
                                            mybir.AluOpType.is_ge,
                                            mybir.AluOpType.mult)
                    nc.vector.tensor_tensor(acc[:], acc[:], tmp[:],
                                            mybir.AluOpType.add)
                y_t = ytsp.tile([P, jg, OSH], bf16, name="y_t", tag=f"yts{g}")
                nc.vector.tensor_scalar(y_t[:], acc[:], float(cvals[0]), None,
                                        mybir.AluOpType.add)
                yts.append(y_t)

            # ---- Stage 2: W^T[i_blk][p_i, o] = sum_j Pi[j,i] * yts[j,o] ---
            for ig in range(nig):
                pi_ts = []
                for k in range(igrp):
                    i_blk = ig * igrp + k
                    pi_t = pip.tile([P, NJ, P], bf16, name="pi_t", tag="pi")
                    nc.sync.dma_start(
                        pi_t[:],
                        gathPi[i_blk].rearrange("(a p) i -> p a i", p=P),
                    )
                    pi_ts.append(pi_t)
                ps = [
                    wpsum.tile([P, OSH], f32, name="wps", tag=f"wps{k}")
                    for k in range(igrp)
                ]
                for j in range(NJ):
                    for k in range(igrp):
                        nc.tensor.matmul(
                            ps[k][:], pi_ts[k][:, j, :],
                            yts[j // jg][:, j % jg, :],
                            start=(j == 0), stop=(j == NJ - 1),
                        )
                for k in range(igrp):
                    i_blk = ig * igrp + k
                    wt_t = wtp.tile([P, OSH], bf16, name="wt_t", tag="wt")
                    nc.scalar.copy(wt_t[:], ps[k][:])
                    for q in range(4):
                        nc.scalar.dma_start(
                            wtbq[q][i_blk], wt_t[:, q * P:(q + 1) * P]
                        )

            # ---- Stage 3: AllGather W^T in 4 o-quarter chunks -------------
            for q in range(4):
                nc.gpsimd.collective_compute(
                    "AllGather", mybir.AluOpType.bypass, replica_groups=rg,
                    ins=[wtbq[q][:]], outs=[gWTq[q][:]],
                )

            # ---- Stage 4: outT[o, t] = rn[o]*sum_i W^T[i,o]*xT[i,t]+b[o] --
            for q in range(4):
                for s in range(NCORES):
                    ob = s * 4 + q
                    wti = wt3p.tile([P, NI, P], bf16, name="wti", tag="wti")
                    nc.sync.dma_start(
                        wti[:], gWTq[q][s].rearrange("b p o -> p b o")
                    )
                    for tb in range(ntc):
                        mp = mpsum.tile([P, tch], f32, name="mp", tag="mp")
                        for ib in range(NI):
                            nc.tensor.matmul(
                                mp[:], wti[:, ib, :],
                                xt[:, ib, tb * tch:(tb + 1) * tch],
                                start=(ib == 0), stop=(ib == NI - 1),
                            )
                        o_t = outp.tile([P, tch], f32, name="o_t", tag="out")
                        nc.vector.tensor_scalar(
                            o_t[:], mp[:], rn_sb[:, ob:ob + 1],
                            bias_sb[:, ob:ob + 1],
                            mybir.AluOpType.mult, mybir.AluOpType.add,
                        )
                        nc.scalar.dma_start(
                            outT_d[ob * P:(ob + 1) * P,
                                   tb * tch:(tb + 1) * tch],
                            o_t[:],
                        )
    nc.compile()
    return nc


def _prep_inputs(x, indices, Pi, row_norms, bias):
    """Host-side layout prep + sharding. Returns list of per-core in_maps."""
    import ml_dtypes

    bf16 = ml_dtypes.bfloat16
    fp8 = ml_dtypes.float8_e4m3

    xT = np.ascontiguousarray(
        np.asarray(x, np.float32).reshape(T, IN).T
    ).astype(bf16)                                    # (IN, T)
    piR = np.ascontiguousarray(
        np.asarray(Pi, np.float32).astype(bf16).reshape(IN, NI, P)
        .transpose(1, 0, 2)
    )                                                 # (NI, IN_j, P_i)
    idxT = np.ascontiguousarray(np.asarray(indices).T).astype(fp8)  # (IN, OUT)
    rn = np.ascontiguousarray(np.asarray(row_norms, np.float32))
    bs = np.ascontiguousarray(np.asarray(bias, np.float32))

    in_maps = []
    for c in range(NCORES):
        in_maps.append({
            "xT": np.ascontiguousarray(xT[:, c * TSH:(c + 1) * TSH]),
            "piC": np.ascontiguousarray(piR[c * 4:(c + 1) * 4]),
            "idx8": np.ascontiguousarray(idxT[:, c * OSH:(c + 1) * OSH]),
            "rn": rn,
            "bias": bs,
        })
    return in_maps


def _get_nc(centroids):
    key = np.asarray(centroids, np.float32).tobytes()
    nc = _NC_CACHE.get(key)
    if nc is None:
        cvals = [float(v) for v in np.asarray(centroids, np.float32)]
        assert len(cvals) == 16
        nc = build_nc(cvals)
        _NC_CACHE.clear()
        _NC_CACHE[key] = nc
    return nc


def kernel(x, indices, centroids, Pi, row_norms, bias):
    from concourse.bass_utils import run_bass_kernel_spmd

    nc = _get_nc(centroids)
    in_maps = _prep_inputs(x, indices, Pi, row_norms, bias)
    res = run_bass_kernel_spmd(nc, in_maps, list(range(NCORES)))
    shards = [np.asarray(res.results[c]["outT"]) for c in range(NCORES)]
    full = np.concatenate(shards, axis=1)             # (OUT, T)
    out = np.ascontiguousarray(full.T).reshape(B, S, OUT)
    return out.astype(np.float32)


# revision 4
# speedup vs baseline: 1.2217x; 1.2217x over previous
"""Trainium2 Bass kernel for CompressedLinear (VQ codebook linear layer).

Computes: out = x @ W^T + bias, where
  W = (centroids[indices] @ Pi) * row_norms[:, None]

Upload-minimizing SPMD layout (8 cores):
  - x token-sharded:  core c gets xT[:, c*1024:(c+1)*1024]   (8 MB bf16)
  - Pi column-sharded: core c gets Pi[:, c*512:(c+1)*512]    (4 MB bf16)
  - indices o-sharded: core c gets idxT[:, c*512:(c+1)*512]  (2 MB fp8)
  - rn/bias replicated (32 KB); output stored bf16 (8 MB/core readback)
Device pipeline per core:
  0. AllGather Pi column shards (4 chunks) -> full Pi on-chip.
  1. DVE gather yts[j,o] = centroids[idxT[j,o]] for the core's o-shard via
     fused custom-DVE pair ops (2 codebook entries per instruction).
  2. PE: W^T[i, o_shard] = sum_j Pi[j,i] * yts[j,o], i-blocks paced by the
     arriving Pi chunks.
  3. AllGather W^T o-shards in 4 o-quarter chunks -> full W^T.
  4. PE: outT[o, t_c] = rn[o] * sum_i W^T[i,o] * xT[i,t_c] + bias[o]
     for ALL output features o, local tokens t_c; o-blocks paced by the
     arriving W^T quarters.
Host reassembles the 8 token-shards of outT.
"""

import numpy as np

# Problem geometry (hardcoded per contract)
OUT, IN = 4096, 4096
B, S = 4, 2048
T = B * S          # 8192 tokens
NCORES = 8
P = 128            # partitions
TSH = T // NCORES  # 1024 tokens per core
OSH = OUT // NCORES  # 512 out-features per core (gather/W stage)
NI = IN // P       # 32 i-blocks
NJ = IN // P       # 32 j-blocks
NOB = OUT // P     # 32 o-blocks (stage 4)

_NC_CACHE = {}
_DVE_OPS = None


def _register_dve_ops():
    """Register the two fused VQ-gather ops in dve_ops.OPS (idempotent)."""
    global _DVE_OPS
    if _DVE_OPS is not None:
        return _DVE_OPS
    import concourse.dve_ops as dvo
    from concourse.dve_spec import Spec, Src0, Src1, C0, C1, C2, One, eq, lower
    from concourse.dve_uop import DveOpSpec

    existing = {op.name: op for op in dvo.OPS}
    if "VQ_PAIR" in existing:
        _DVE_OPS = {k: existing[k] for k in ("VQ_PAIR", "VQ_ACC2")}
        return _DVE_OPS

    ver = "v3"  # TRN2

    def mk(name, spec, rd1):
        opcode = dvo._CUSTOM_DVE_ROW_BASE + len(dvo.OPS)
        dvo._SUB_OPCODE_FOR_NAME[name] = opcode
        s = DveOpSpec(name=name, opcode=opcode, uops=lower(spec, ver=ver), rd1_en=rd1)
        op = dvo.DveOp(name, spec, subdim=False, uops_sha={ver: s.sha(ver)})
        dvo.OPS.append(op)
        dvo.CUSTOM_DVE_SPECS[name] = spec
        return op

    # out = (idx==imm2)*s0 + (idx==imm2+1)*s1
    pair = mk(
        "VQ_PAIR",
        Spec(
            body=eq(Src0, C2) * C0 + eq(Src0, C2 + One) * C1,
            reference=lambda in0, in1, s0, s1, imm2: (
                (in0 == imm2) * s0 + (in0 == imm2 + 1) * s1
            ).astype(np.float32),
        ),
        False,
    )
    # out = acc + (idx==imm2)*s0 + (idx==imm2+1)*s1
    acc = mk(
        "VQ_ACC2",
        Spec(
            body=Src1 + eq(Src0, C2) * C0 + eq(Src0, C2 + One) * C1,
            reference=lambda in0, in1, s0, s1, imm2: (
                in1 + (in0 == imm2) * s0 + (in0 == imm2 + 1) * s1
            ).astype(np.float32),
        ),
        True,
    )
    _DVE_OPS = {"VQ_PAIR": pair, "VQ_ACC2": acc}
    return _DVE_OPS


def build_nc(cvals):
    """Build the SPMD Bass program. cvals: 16 python floats (codebook)."""
    import concourse.bacc as bacc
    import concourse.mybir as mybir
    from concourse.tile import TileContext

    ops = _register_dve_ops()

    f32 = mybir.dt.float32
    bf16 = mybir.dt.bfloat16
    fp8 = mybir.dt.float8e4

    rg = [list(range(NCORES))]
    jg = 4             # j-blocks per gather group
    ngr = NJ // jg     # 8 gather groups
    igrp = 4           # i-blocks per stage-2 psum group
    tch = 512          # stage-4 token chunk
    ntc = TSH // tch   # 2

    nc = bacc.Bacc()
    xT_d = nc.dram_tensor("xT", [IN, TSH], bf16, kind="ExternalInput")
    piC_d = nc.dram_tensor("piC", [4, IN, P], bf16, kind="ExternalInput")
    idx_d = nc.dram_tensor("idx8", [IN, OSH], fp8, kind="ExternalInput")
    rn_d = nc.dram_tensor("rn", [OUT], f32, kind="ExternalInput")
    bias_d = nc.dram_tensor("bias", [OUT], f32, kind="ExternalInput")
    outT_d = nc.dram_tensor("outT", [OUT, TSH], bf16, kind="ExternalOutput")

    # Pi AG chunks: chunk qc carries each core's qc-th i-block
    piBq = [nc.dram_tensor(f"piB{q}", [IN, P], bf16) for q in range(4)]
    gPiQ = [
        nc.dram_tensor(f"gPi{q}", [NCORES, IN, P], bf16, addr_space="Shared")
        for q in range(4)
    ]
    # W^T AG chunks: chunk oq carries each core's o-quarter oq
    wtbq = [nc.dram_tensor(f"wtb{q}", [NI, P, P], bf16) for q in range(4)]
    gWTq = [
        nc.dram_tensor(f"gWT{q}", [NCORES, NI, P, P], bf16, addr_space="Shared")
        for q in range(4)
    ]

    with TileContext(nc) as tc:
        with (
            tc.tile_pool(name="constp", bufs=1) as constp,
            tc.tile_pool(name="idxp", bufs=2) as idxp,
            tc.tile_pool(name="pingp", bufs=2) as pingp,
            tc.tile_pool(name="ytsp", bufs=1) as ytsp,
            tc.tile_pool(name="pip", bufs=5) as pip,
            tc.tile_pool(name="wtp", bufs=4) as wtp,
            tc.tile_pool(name="xtp", bufs=1) as xtp,
            tc.tile_pool(name="wt3p", bufs=3) as wt3p,
            tc.tile_pool(name="outp", bufs=4) as outp,
            tc.tile_pool(name="wpsum", bufs=1, space="PSUM") as wpsum,
            tc.tile_pool(name="mpsum", bufs=3, space="PSUM") as mpsum,
        ):
            rn_sb = constp.tile([P, NOB], f32, name="rn_sb")
            nc.sync.dma_start(rn_sb[:], rn_d.rearrange("(b p) -> p b", p=P))
            bias_sb = constp.tile([P, NOB], f32, name="bias_sb")
            nc.sync.dma_start(bias_sb[:], bias_d.rearrange("(b p) -> p b", p=P))

            # ---- Stage 0: AllGather Pi in 4 chunks (starts immediately) ---
            for q in range(4):
                nc.gpsimd.dma_start(piBq[q][:], piC_d[q])
                nc.gpsimd.collective_compute(
                    "AllGather", mybir.AluOpType.bypass, replica_groups=rg,
                    ins=[piBq[q][:]], outs=[gPiQ[q][:]],
                )

            # Resident xT for stage 4 (local input; scalar queue is idle now)
            xt = xtp.tile([P, NI, TSH], bf16, name="xt")
            nc.scalar.dma_start(
                xt[:], xT_d.rearrange("(a p) t -> p a t", p=P)
            )

            # ---- Stage 1: gather yts[g][p, (b o)] = centroids[idxT] -------
            # Fused pair ops: 2 codebook entries per instruction, 8 per tile.
            yts = []
            for g in range(ngr):
                idx_t = idxp.tile([P, jg * OSH], fp8, name="idx_t", tag="idx")
                for b in range(jg):
                    jb = g * jg + b
                    nc.sync.dma_start(
                        idx_t[:, b * OSH:(b + 1) * OSH],
                        idx_d[jb * P:(jb + 1) * P, :],
                    )
                a = pingp.tile([P, jg * OSH], bf16, name="ya", tag="ya")
                nc.vector._custom_dve(
                    ops["VQ_PAIR"], out=a[:], in0=idx_t[:],
                    s0=float(cvals[0]), s1=float(cvals[1]), imm2=0.0,
                )
                b = pingp.tile([P, jg * OSH], bf16, name="yb", tag="yb")
                for k in range(1, 7):
                    src, dst = (a, b) if k % 2 == 1 else (b, a)
                    nc.vector._custom_dve(
                        ops["VQ_ACC2"], out=dst[:], in0=idx_t[:], in1=src[:],
                        s0=float(cvals[2 * k]), s1=float(cvals[2 * k + 1]),
                        imm2=float(2 * k),
                    )
                y_t = ytsp.tile([P, jg * OSH], bf16, name="y_t", tag=f"yts{g}")
                nc.vector._custom_dve(
                    ops["VQ_ACC2"], out=y_t[:], in0=idx_t[:], in1=a[:],
                    s0=float(cvals[14]), s1=float(cvals[15]), imm2=14.0,
                )
                yts.append(y_t)

            # ---- Stage 2: W^T[i_blk][p_i, o] = sum_j Pi[j,i] * yts[j,o] ---
            # i_blk = 4*c + qc arrives with Pi chunk qc (rank c's block)
            for qc in range(4):
                for half in range(2):
                    pi_ts = []
                    for k in range(igrp):
                        c = half * igrp + k
                        pi_t = pip.tile([P, NJ, P], bf16, name="pi_t", tag="pi")
                        nc.sync.dma_start(
                            pi_t[:],
                            gPiQ[qc][c].rearrange("(a p) i -> p a i", p=P),
                        )
                        pi_ts.append(pi_t)
                    ps = [
                        wpsum.tile([P, OSH], f32, name="wps", tag=f"wps{k}")
                        for k in range(igrp)
                    ]
                    for j in range(NJ):
                        for k in range(igrp):
                            nc.tensor.matmul(
                                ps[k][:], pi_ts[k][:, j, :],
                                yts[j // jg][:, (j % jg) * OSH:
                                             (j % jg + 1) * OSH],
                                start=(j == 0), stop=(j == NJ - 1),
                            )
                    for k in range(igrp):
                        i_blk = 4 * (half * igrp + k) + qc
                        wt_t = wtp.tile([P, OSH], bf16, name="wt_t", tag="wt")
                        nc.scalar.copy(wt_t[:], ps[k][:])
                        for oq in range(4):
                            nc.scalar.dma_start(
                                wtbq[oq][i_blk], wt_t[:, oq * P:(oq + 1) * P]
                            )

            # ---- Stage 3: AllGather W^T in 4 o-quarter chunks -------------
            for oq in range(4):
                nc.gpsimd.collective_compute(
                    "AllGather", mybir.AluOpType.bypass, replica_groups=rg,
                    ins=[wtbq[oq][:]], outs=[gWTq[oq][:]],
                )

            # ---- Stage 4: outT[o, t] = rn[o]*sum_i W^T[i,o]*xT[i,t]+b[o] --
            for oq in range(4):
                for s in range(NCORES):
                    ob = s * 4 + oq
                    wti = wt3p.tile([P, NI, P], bf16, name="wti", tag="wti")
                    nc.sync.dma_start(
                        wti[:], gWTq[oq][s].rearrange("b p o -> p b o")
                    )
                    for tb in range(ntc):
                        mp = mpsum.tile([P, tch], f32, name="mp", tag="mp")
                        for ib in range(NI):
                            nc.tensor.matmul(
                                mp[:], wti[:, ib, :],
                                xt[:, ib, tb * tch:(tb + 1) * tch],
                                start=(ib == 0), stop=(ib == NI - 1),
                            )
                        o_t = outp.tile([P, tch], bf16, name="o_t", tag="out")
                        nc.vector.tensor_scalar(
                            o_t[:], mp[:], rn_sb[:, ob:ob + 1],
                            bias_sb[:, ob:ob + 1],
                            mybir.AluOpType.mult, mybir.AluOpType.add,
                        )
                        nc.scalar.dma_start(
                            outT_d[ob * P:(ob + 1) * P,
                                   tb * tch:(tb + 1) * tch],
                            o_t[:],
                        )
    nc.compile()
    return nc


def _prep_inputs(x, indices, Pi, row_norms, bias):
    """Host-side layout prep + sharding. Returns list of per-core in_maps."""
    import ml_dtypes

    bf16 = ml_dtypes.bfloat16
    fp8 = ml_dtypes.float8_e4m3

    xT = np.ascontiguousarray(
        np.asarray(x, np.float32).reshape(T, IN).T
    ).astype(bf16)                                    # (IN, T)
    piR = np.ascontiguousarray(
        np.asarray(Pi, np.float32).astype(bf16).reshape(IN, NI, P)
        .transpose(1, 0, 2)
    )                                                 # (NI, IN_j, P_i)
    idxT = np.ascontiguousarray(np.asarray(indices).T).astype(fp8)  # (IN, OUT)
    rn = np.ascontiguousarray(np.asarray(row_norms, np.float32))
    bs = np.ascontiguousarray(np.asarray(bias, np.float32))

    in_maps = []
    for c in range(NCORES):
        # piC[q] = global i-block 4*c + q (chunk q of the Pi AllGather)
        piC = np.ascontiguousarray(piR[4 * c:4 * (c + 1)])
        in_maps.append({
            "xT": np.ascontiguousarray(xT[:, c * TSH:(c + 1) * TSH]),
            "piC": piC,
            "idx8": np.ascontiguousarray(idxT[:, c * OSH:(c + 1) * OSH]),
            "rn": rn,
            "bias": bs,
        })
    return in_maps


def _get_nc(centroids):
    key = np.asarray(centroids, np.float32).tobytes()
    nc = _NC_CACHE.get(key)
    if nc is None:
        cvals = [float(v) for v in np.asarray(centroids, np.float32)]
        assert len(cvals) == 16
        nc = build_nc(cvals)
        _NC_CACHE.clear()
        _NC_CACHE[key] = nc
    return nc


def kernel(x, indices, centroids, Pi, row_norms, bias):
    from concourse.bass_utils import run_bass_kernel_spmd

    nc = _get_nc(centroids)
    in_maps = _prep_inputs(x, indices, Pi, row_norms, bias)
    res = run_bass_kernel_spmd(nc, in_maps, list(range(NCORES)))
    shards = [np.asarray(res.results[c]["outT"]) for c in range(NCORES)]
    full = np.concatenate(shards, axis=1)             # (OUT, T) bf16
    out = np.ascontiguousarray(full.T).reshape(B, S, OUT)
    return out.astype(np.float32)
